# revision 42
# baseline (speedup 1.0000x reference)
"""FootballGCN (3x GCNConv + mean-pool + MLP) on 8 TRN2 NeuronCores.

Self-contained: takes full inputs, shards internally, runs a Bass/Tile SPMD
kernel via run_bass_kernel_spmd, returns the full (B, 1) output.

Strategy (dst-sharded message passing, feature-major on-chip layout):
  - nodes packed into 8 cores x 12544 local slots (12500 real + pad)
  - per layer: table t = dis * (h @ W) built per 128-node block via one
    matmul (lhsT = feature-major h block -> node-major psum, no transpose),
    quantized f16, duplicated to 256B rows, AllGathered to every core's HBM
  - edge pass: (g, blk)-major slot stream with core-invariant per-(g,blk)
    slot counts; chunks of 128 slots cut independently of block boundaries;
    per (group, step) one dma_gather of up to CH chunks, with desc-gen
    parallelized across the 4 SWDGE queues (queue_num=g selects the Q7 DSP
    pair in ucode, so 4 gathers' descriptor generation runs concurrently)
  - selection matrices on DVE via is_equal(dl, iota) where
    dl = dcol + 128*(blk != bfirst(chunk)); PE matmul (lhsT=msg, rhs=Sel)
    accumulates agg[64, 128] per block in PSUM; self-loops are folded in as
    one transpose-matmul of the local t block (lhsT=tst, rhs=I128)
  - epilogue relu(dis*agg + b) on DVE+ACT
  - pooling via segsel matmuls, AllReduce, tiny MLP, output z
"""
import numpy as np

import concourse.bass as bass
import concourse.mybir as mybir
import concourse.tile as tile
from concourse import bacc as bacc_mod
from concourse.bass_utils import run_bass_kernel_spmd

F16 = mybir.dt.float16
F32 = mybir.dt.float32
F8 = mybir.dt.float8e4
I16 = mybir.dt.int16

# ---- problem dims (hardcoded per spec) ----
N = 100000
E = 3200000
B = 128
IN_C, HID = 128, 64
NCORES = 8
NREAL = 12500
NLOC = 12544                 # 98 * 128
NBLK = NLOC // 128           # 98
NTOT = NCORES * NLOC         # 100352
NGRP = 4
GRP = NTOT // NGRP           # 25088
NHALF = NLOC // 2            # 6272 rows = blocks 0-48
CH = 21                      # chunks per gather


def _preprocess(edge_index, batch):
    src_g = np.asarray(edge_index[0], np.int64)
    dst_g = np.asarray(edge_index[1], np.int64)
    loops = np.arange(N, dtype=np.int64)
    dst_all = np.concatenate([dst_g, loops])
    deg = np.bincount(dst_all, minlength=N).astype(np.float64)
    dis = (1.0 / np.sqrt(np.maximum(deg, 1.0))).astype(np.float32)

    # snake-balanced node -> packed-slot assignment: deal nodes (sorted by
    # in-degree desc) across all NCORES*NBLK blocks so per-(core,g,blk)
    # edge counts equalize -> less cross-core max padding.
    nblk_all = NCORES * NBLK
    order = np.argsort(-deg, kind="stable")
    pos = np.arange(N)
    cyc, r = pos // nblk_all, pos % nblk_all
    blk_of = np.where(cyc % 2 == 0, r, nblk_all - 1 - r)
    rank_of = cyc
    perm = np.empty(N, dtype=np.int64)
    perm[order] = (blk_of // NBLK) * NLOC + (blk_of % NBLK) * 128 + rank_of
    assert rank_of.max() < 128

    # gather groups are (src table-half, src core-quad): the table is
    # AllGathered in two halves (tfA = local rows [0, NHALF), tfB = rest),
    # so a group must be one contiguous 25088-row window of tfA or tfB.
    # Group membership is invariant under the within-pair LPT below.
    def grp_lidx(p):
        c, loc = p // NLOC, p % NLOC
        g = (loc >= NHALF) * 2 + (c >= 4)
        return g, (c % 4) * NHALF + (loc % NHALF)

    # LPT within each (core-pair, block-col): rebalance nodes between the
    # two cores to equalize per-(g, blk) in-edge counts across cores.
    grp_node0 = grp_lidx(perm)[0]
    vg = np.zeros((N, NGRP), np.int32)
    np.add.at(vg, (dst_g, grp_node0[src_g]), 1)
    core_of = perm // NLOC
    blk_idx = (perm % NLOC) // 128
    for p in range(4):
        for b in range(NBLK):
            sel = np.flatnonzero(
                ((core_of == 2 * p) | (core_of == 2 * p + 1)) & (blk_idx == b))
            v = vg[sel].astype(np.int64)
            o2 = np.argsort(-v.sum(1), kind="stable")
            sumA = np.zeros(NGRP, np.int64)
            sumB = np.zeros(NGRP, np.int64)
            nA = nB = 0
            sideA = np.zeros(len(sel), bool)
            for i in o2:
                mA = np.maximum(sumA + v[i], sumB).max()
                mB = np.maximum(sumA, sumB + v[i]).max()
                if (mA <= mB and nA < 128) or nB >= 128:
                    sideA[i] = True
                    sumA += v[i]
                    nA += 1
                else:
                    sumB += v[i]
                    nB += 1
            ranksA = np.flatnonzero(sideA)
            ranksB = np.flatnonzero(~sideA)
            newpos = np.empty(len(sel), np.int64)
            newpos[ranksA] = 2 * p * NLOC + b * 128 + np.arange(len(ranksA))
            newpos[ranksB] = ((2 * p + 1) * NLOC + b * 128
                              + np.arange(len(ranksB)))
            perm[sel] = newpos

    # self-loops are NOT in the stream (folded into the per-block
    # transpose-matmul of the local t block instead)
    src_p = perm[src_g]
    dst_p = perm[dst_g]

    core = dst_p // NLOC
    blk = (dst_p % NLOC) // 128
    dcol = dst_p % 128
    grp, lidx = grp_lidx(src_p)

    key = (core * NGRP + grp) * NBLK + blk
    cnt = np.bincount(key, minlength=NCORES * NGRP * NBLK).reshape(
        NCORES, NGRP, NBLK)
    m = cnt.max(axis=0)                      # (NGRP, NBLK) core-invariant
    CU = int(np.ceil(m.sum(axis=1).max() / 128))
    SGRP = CU * 128
    S = NGRP * SGRP

    off = np.zeros((NGRP, NBLK), np.int64)
    off[:, 1:] = np.cumsum(m[:, :-1], axis=1)

    # first/last block of each (g, chunk)
    seg_end = np.cumsum(m, axis=1)           # (NGRP, NBLK) slot end per block
    bfirst = np.zeros((NGRP, CU), np.int64)
    blast = np.zeros((NGRP, CU), np.int64)
    for g in range(NGRP):
        bfirst[g] = np.searchsorted(seg_end[g], np.arange(CU) * 128,
                                    side="right")
        bfirst[g] = np.minimum(bfirst[g], NBLK - 1)
        blast[g] = np.searchsorted(seg_end[g], np.arange(1, CU + 1) * 128 - 1,
                                   side="right")
        blast[g] = np.minimum(blast[g], NBLK - 1)

    # boundary chunks (span a block boundary) need a second sel built from
    # dlB = dl - 128; bnd_ord[g, ch] = ordinal into the packed dlB table
    bnd_ord = np.full((NGRP, CU), -1, np.int64)
    nbnd = 0
    for g in range(NGRP):
        for ch in range(CU):
            if bfirst[g, ch] != blast[g, ch]:
                bnd_ord[g, ch] = nbnd
                nbnd += 1

    # slot assignment: sort edges by (core, g, blk, lidx); lidx order gives
    # ascending HBM addresses within each run for better row locality
    o = np.lexsort((lidx, blk, grp, core))
    core_s, grp_s, blk_s, dcol_s, lidx_s = (
        core[o], grp[o], blk[o], dcol[o], lidx[o])
    keyo = (core_s * NGRP + grp_s) * NBLK + blk_s
    is_start = np.ones(len(keyo), bool)
    is_start[1:] = keyo[1:] != keyo[:-1]
    run_start = np.flatnonzero(is_start)
    run_id = np.cumsum(is_start) - 1
    run_pos = np.arange(len(keyo)) - run_start[run_id]
    slot = grp_s * SGRP + off[grp_s, blk_s] + run_pos

    idx_stream = np.zeros((NCORES, S), np.int16)
    dl_stream = np.full((NCORES, S), -1.0, np.float16)
    idx_stream[core_s, slot] = lidx_s.astype(np.int16)
    chunk_of = (slot % SGRP) // 128
    second = (blk_s != bfirst[grp_s, chunk_of]).astype(np.int64)
    dl_stream[core_s, slot] = (dcol_s + 128 * second).astype(np.float16)

    # per-block job chunk ranges (group-local chunk indices)
    jobs = np.zeros((NBLK, NGRP, 2), np.int64)
    for b in range(NBLK):
        for g in range(NGRP):
            s0, s1 = off[g, b], off[g, b] + m[g, b]
            jobs[b, g, 0] = s0 // 128
            jobs[b, g, 1] = (s1 + 127) // 128

    NSTEP = (CU + CH - 1) // CH
    done_step = np.zeros(NBLK, np.int64)
    for b in range(NBLK):
        endc = jobs[b, :, 1].max()
        done_step[b] = (int(endc) + CH - 1) // CH - 1 if endc > 0 else 0

    # host-packed dl tables: dl per chunk, and dl-128 for boundary chunks
    dl_cols = dl_stream.reshape(NCORES, S // 128, 128)  # (cores, Ctot, 128)
    dlB = np.zeros((NCORES, max(nbnd, 1), 128), np.float16)
    for g in range(NGRP):
        for ch in range(CU):
            j = bnd_ord[g, ch]
            if j >= 0:
                dlB[:, j, :] = (
                    dl_cols[:, g * CU + ch, :].astype(np.float32) - 128.0
                ).astype(np.float16)

    meta = dict(CU=CU, S=S, jobs=jobs, bfirst=bfirst, NSTEP=NSTEP,
                done_step=done_step, bnd_ord=bnd_ord, nbnd=max(nbnd, 1))
    return dis, meta, idx_stream, dl_stream, dlB, perm


def _build_nc(meta):
    CU, S = meta["CU"], meta["S"]
    jobs, bfirst = meta["jobs"], meta["bfirst"]
    NSTEP, done_step = meta["NSTEP"], meta["done_step"]
    bnd_ord, NBND = meta["bnd_ord"], meta["nbnd"]
    SGRP = CU * 128
    Ctot = S // 128

    # per (g, step): [first boundary ordinal, count] in the step's chunk range
    bnd_rng = np.zeros((NGRP, NSTEP, 2), np.int64)
    for g in range(NGRP):
        for st in range(NSTEP):
            ords = [bnd_ord[g, ch]
                    for ch in range(st * CH, min((st + 1) * CH, CU))
                    if bnd_ord[g, ch] >= 0]
            if ords:
                bnd_rng[g, st] = (ords[0], len(ords))

    # 4 SWDGE queues: dma_gather desc-gen runs on the Q7 DSP pair selected
    # by queue_num (ucode: cpu_id/2 == queue_num), so the 4 per-step group
    # gathers generate descriptors concurrently on disjoint DSP pairs.
    nc = bacc_mod.Bacc(num_swdge_queues=4)

    xT = nc.declare_dram_parameter("xT", [IN_C, NLOC], F16, isOutput=False)
    disN = nc.declare_dram_parameter("disN", [NLOC, 1], F32, isOutput=False)
    disT = nc.declare_dram_parameter("disT", [1, NLOC], F16, isOutput=False)
    W0 = nc.declare_dram_parameter("W0", [IN_C, HID], F16, isOutput=False)
    W1 = nc.declare_dram_parameter("W1", [HID, HID], F16, isOutput=False)
    W2 = nc.declare_dram_parameter("W2", [HID, HID], F16, isOutput=False)
    b0 = nc.declare_dram_parameter("b0", [HID, 1], F32, isOutput=False)
    b1 = nc.declare_dram_parameter("b1", [HID, 1], F32, isOutput=False)
    b2 = nc.declare_dram_parameter("b2", [HID, 1], F32, isOutput=False)
    Wm1 = nc.declare_dram_parameter("Wm1", [HID, HID // 2], F32, isOutput=False)
    bm1 = nc.declare_dram_parameter("bm1", [HID // 2, 1], F32, isOutput=False)
    Wm2 = nc.declare_dram_parameter("Wm2", [HID // 2, 1], F32, isOutput=False)
    bm2 = nc.declare_dram_parameter("bm2", [1, 1], F32, isOutput=False)
    idx16 = nc.declare_dram_parameter("idx16", [128, S // 16], I16, isOutput=False)
    dl16 = nc.declare_dram_parameter("dl16", [128, Ctot], F16, isOutput=False)
    dlB16 = nc.declare_dram_parameter("dlB16", [128, NBND], F16, isOutput=False)
    segsel = nc.declare_dram_parameter("segsel", [NLOC, B], F16, isOutput=False)
    invcnt = nc.declare_dram_parameter("invcnt", [B, 1], F32, isOutput=False)
    iota128 = nc.declare_dram_parameter("iota128", [128, 256], F16, isOutput=False)
    ident64 = nc.declare_dram_parameter("ident64", [HID, HID], F16, isOutput=False)
    ident128 = nc.declare_dram_parameter("ident128", [128, 128], F16, isOutput=False)
    identB = nc.declare_dram_parameter("identB", [B, B], F32, isOutput=False)
    z = nc.declare_dram_parameter("z", [1, B], F32, isOutput=True)

    # half-table staging + AllGather outputs, double-buffered by layer parity
    t_lA = [nc.dram_tensor(f"t_lA{p}", [NHALF, 128], F16) for p in range(2)]
    t_lB = [nc.dram_tensor(f"t_lB{p}", [NHALF, 128], F16) for p in range(2)]
    tfA = [nc.dram_tensor(f"tfA{p}", [NCORES * NHALF, 128], F16,
                          addr_space="Shared") for p in range(2)]
    tfB = [nc.dram_tensor(f"tfB{p}", [NCORES * NHALF, 128], F16,
                          addr_space="Shared") for p in range(2)]
    pool_in = nc.dram_tensor("pool_in", [B, HID], F32)
    pool_out = nc.dram_tensor("pool_out", [B, HID], F32, addr_space="Shared")

    groups = [list(range(NCORES))]

    with tile.TileContext(nc) as tc:
        with (
            tc.tile_pool(name="const", bufs=1) as constp,
            tc.tile_pool(name="hT", bufs=1) as hTp,
            tc.tile_pool(name="tstage", bufs=1) as tstp,
            tc.tile_pool(name="xblk", bufs=3) as xblkp,
            tc.tile_pool(name="idx", bufs=2) as idxp,
            tc.tile_pool(name="msg", bufs=2) as msgp,
            tc.tile_pool(name="sel", bufs=2) as selp,
            tc.tile_pool(name="eps", bufs=3) as epsp,
            tc.tile_pool(name="psA", bufs=2, space="PSUM") as psA,
            tc.tile_pool(name="psB", bufs=2, space="PSUM") as psB,
            tc.tile_pool(name="psPool", bufs=1, space="PSUM") as psPoolp,
        ):
            disN_sb = constp.tile([128, NBLK], F32)
            nc.sync.dma_start(
                out=disN_sb[:], in_=disN.rearrange("(b p) o -> p (b o)", p=128))
            disTfull = constp.tile([HID, NLOC], F16)
            nc.sync.dma_start(
                out=disTfull[:].unsqueeze(1),
                in_=disT[:, :].partition_broadcast(HID))
            W0_sb = constp.tile([IN_C, HID], F16)
            nc.sync.dma_start(out=W0_sb[:], in_=W0[:])
            xfull = constp.tile([IN_C, NLOC], F16)
            nc.sync.dma_start(out=xfull[:], in_=xT[:])
            W1_sb = constp.tile([HID, HID], F16)
            nc.sync.dma_start(out=W1_sb[:], in_=W1[:])
            W2_sb = constp.tile([HID, HID], F16)
            nc.sync.dma_start(out=W2_sb[:], in_=W2[:])
            bias_sb = constp.tile([HID, 3], F32)
            for i, bb in enumerate([b0, b1, b2]):
                nc.sync.dma_start(out=bias_sb[:, i:i + 1], in_=bb[:])
            iota_sb = constp.tile([128, 256], F16)
            nc.sync.dma_start(out=iota_sb[:], in_=iota128[:])
            dl_sb = constp.tile([128, Ctot], F16)
            nc.sync.dma_start(out=dl_sb[:], in_=dl16[:])
            dlB_sb = constp.tile([128, NBND], F16)
            nc.sync.dma_start(out=dlB_sb[:], in_=dlB16[:])
            ident64_sb = constp.tile([HID, HID], F16)
            nc.sync.dma_start(out=ident64_sb[:], in_=ident64[:])
            ident128_sb = constp.tile([128, 128], F16)
            nc.sync.dma_start(out=ident128_sb[:], in_=ident128[:])
            identB_sb = constp.tile([B, B], F32)
            nc.sync.dma_start(out=identB_sb[:], in_=identB[:])
            invcnt_sb = constp.tile([B, 1], F32)
            nc.sync.dma_start(out=invcnt_sb[:], in_=invcnt[:])
            mlpw_sb = constp.tile([HID, HID // 2 + 1], F32)
            nc.sync.dma_start(out=mlpw_sb[:, :HID // 2], in_=Wm1[:])
            nc.sync.dma_start(out=mlpw_sb[:HID // 2, HID // 2:], in_=Wm2[:])
            bm_sb = constp.tile([HID // 2, 2], F32)
            nc.sync.dma_start(out=bm_sb[:, 0:1], in_=bm1[:])
            nc.sync.dma_start(out=bm_sb[0:1, 1:2], in_=bm2[:])

            hT = hTp.tile([HID, NLOC], F16, tag="hT")

            def stage_ag(par, half, tsrc):
                # stage one table half (duplicated to 256B rows) + AllGather
                t_l = (t_lA if half == 0 else t_lB)[par]
                tf = (tfA if half == 0 else tfB)[par]
                tdst = t_l.rearrange("(b p) e -> p b e", p=128)
                b0 = half * (NBLK // 2)
                nc.sync.dma_start(
                    out=tdst[:, :, 0:HID], in_=tsrc[:, b0:b0 + NBLK // 2, :])
                nc.sync.dma_start(
                    out=tdst[:, :, HID:128], in_=tsrc[:, b0:b0 + NBLK // 2, :])
                nc.gpsimd.collective_compute(
                    "AllGather", mybir.AluOpType.bypass, replica_groups=groups,
                    ins=[t_l[:]], outs=[tf[:]])

            # layer-0 t-table upfront from x; first-half AllGather fires at
            # the midpoint so it overlaps the rest of the build
            tst = tstp.tile([128, NBLK, HID], F16, tag="tstage")
            for b in range(NBLK):
                pt = psA.tile([128, HID], F32, tag="psA")
                nc.tensor.matmul(
                    pt[:], xfull[:, b * 128:(b + 1) * 128], W0_sb[:],
                    start=True, stop=True)
                nc.vector.tensor_tensor(
                    out=tst[:, b, :], in0=pt[:],
                    in1=disN_sb[:, b:b + 1].to_broadcast([128, HID]),
                    op=mybir.AluOpType.mult)
                if b == NBLK // 2 - 1:
                    stage_ag(0, 0, tst)
            stage_ag(0, 1, tst)

            for l in range(3):
                par = l % 2
                tst_next = None
                if l < 2:
                    tst_next = tstp.tile([128, NBLK, HID], F16, tag="tstage")
                else:
                    pool_ps = psPoolp.tile([B, HID], F32)
                    segd = segsel.rearrange("(b p) g -> p b g", p=128)
                Wn_sb = W1_sb if l == 0 else W2_sb

                mts = [[None] * NSTEP for _ in range(NGRP)]
                selAs = [[None] * NSTEP for _ in range(NGRP)]
                selBs = [[None] * NSTEP for _ in range(NGRP)]
                nb = 0
                # issue order: tfA-group gathers of the first PRE steps go
                # first — tfA was AllGathered mid-previous-layer, so these
                # run while this layer's AG2 (tfB) is still completing.
                # PRE=2 keeps every pool's rotation within its buffer count.
                PRE = 2
                plan = ([(s, g) for s in range(min(PRE, NSTEP))
                         for g in range(2)]
                        + [(s, g) for s in range(min(PRE, NSTEP))
                           for g in range(2, NGRP)]
                        + [(s, g) for s in range(PRE, NSTEP)
                           for g in range(NGRP)])
                issued = [0] * NSTEP
                scomp = -1
                for (step, g) in plan:
                    c0 = step * CH
                    c1 = min((step + 1) * CH, CU)
                    nch = c1 - c0
                    sl0 = g * SGRP + c0 * 128
                    nidx = nch * 128
                    idxb = idxp.tile([128, nidx // 16], I16, tag=f"idx{g}")
                    nc.sync.dma_start(
                        out=idxb[:],
                        in_=idx16[:, sl0 // 16:(sl0 + nidx) // 16])
                    mt = msgp.tile([128, nch, 128], F16, tag=f"msg{g}")
                    tf_g = (tfA if g < 2 else tfB)[par]
                    goff = (g % 2) * GRP
                    nc.gpsimd.dma_gather(
                        out_ap=mt[:],
                        in_ap=tf_g[goff:goff + GRP, :],
                        idxs_ap=idxb[:],
                        num_idxs=nidx, num_idxs_reg=nidx, elem_size=128,
                        single_packet=False, queue_num=g)
                    mts[g][step] = mt
                    selA = selp.tile([128, nch, 128], F8, tag=f"selA{g}")
                    nc.vector.tensor_tensor(
                        out=selA[:],
                        in0=dl_sb[:, g * CU + c0:g * CU + c1]
                        .unsqueeze(2).to_broadcast([128, nch, 128]),
                        in1=iota_sb[:, 0:128].unsqueeze(1)
                        .to_broadcast([128, nch, 128]),
                        op=mybir.AluOpType.is_equal)
                    selAs[g][step] = selA
                    o0, nbd = int(bnd_rng[g, step, 0]), int(bnd_rng[g, step, 1])
                    if nbd > 0:
                        selB = selp.tile([128, nbd, 128], F8, tag=f"selB{g}")
                        nc.vector.tensor_tensor(
                            out=selB[:],
                            in0=dlB_sb[:, o0:o0 + nbd]
                            .unsqueeze(2).to_broadcast([128, nbd, 128]),
                            in1=iota_sb[:, 0:128].unsqueeze(1)
                            .to_broadcast([128, nbd, 128]),
                            op=mybir.AluOpType.is_equal)
                        selBs[g][step] = (selB, o0)

                    issued[step] += 1
                    while (scomp + 1 < NSTEP
                           and issued[scomp + 1] == NGRP):
                        scomp += 1
                    while nb < NBLK and done_step[nb] <= scomp:
                        b = nb
                        nb += 1
                        agg = psB.tile([HID, 128], F32, tag="psB")
                        first = True
                        for g in range(NGRP):
                            jc0, jc1 = int(jobs[b, g, 0]), int(jobs[b, g, 1])
                            for ch in range(jc0, jc1):
                                st, col = ch // CH, ch % CH
                                if bfirst[g, ch] == b:
                                    stile = selAs[g][st][:, col, :]
                                else:
                                    sB, o0 = selBs[g][st]
                                    stile = sB[:, int(bnd_ord[g, ch]) - o0, :]
                                nc.tensor.matmul(
                                    agg[:], mts[g][st][:, col, 0:HID],
                                    stile,
                                    start=first, stop=False)
                                first = False
                        # self-loop: agg[f, n] += t_blk[n, f] via transpose
                        nc.tensor.matmul(
                            agg[:], tst[:, b, :], ident128_sb[:],
                            start=first, stop=True)
                        ep = epsp.tile([HID, 128], F32, tag="eps")
                        nc.vector.tensor_tensor(
                            out=ep[:], in0=agg[:],
                            in1=disTfull[:, b * 128:(b + 1) * 128],
                            op=mybir.AluOpType.mult)
                        nc.scalar.activation(
                            out=hT[:, b * 128:(b + 1) * 128], in_=ep[:],
                            func=mybir.ActivationFunctionType.Relu,
                            bias=bias_sb[:, l:l + 1])
                        if l < 2:
                            # next layer's t for this block, overlapped here
                            pt = psA.tile([128, HID], F32, tag="psA")
                            nc.tensor.matmul(
                                pt[:], hT[:, b * 128:(b + 1) * 128], Wn_sb[:],
                                start=True, stop=True)
                            nc.vector.tensor_tensor(
                                out=tst_next[:, b, :], in0=pt[:],
                                in1=disN_sb[:, b:b + 1].to_broadcast([128, HID]),
                                op=mybir.AluOpType.mult)
                            if b == NBLK // 2 - 1:
                                # first-half AllGather for layer l+1 fires
                                # mid-layer and hides under the edge pass
                                stage_ag(1 - par, 0, tst_next)
                        else:
                            # pooling contribution, overlapped here
                            h3t = psA.tile([128, HID], F32, tag="psA")
                            nc.tensor.matmul(
                                h3t[:], hT[:, b * 128:(b + 1) * 128],
                                ident64_sb[:], start=True, stop=True)
                            h3s = epsp.tile([128, HID], F16, tag="h3s")
                            nc.vector.tensor_copy(out=h3s[:], in_=h3t[:])
                            segb = xblkp.tile([128, B], F16, tag="segb")
                            nc.sync.dma_start(out=segb[:], in_=segd[:, b, :])
                            nc.tensor.matmul(
                                pool_ps[:], segb[:], h3s[:],
                                start=(b == 0), stop=(b == NBLK - 1))
                assert nb == NBLK
                if l < 2:
                    stage_ag(1 - par, 1, tst_next)
                    tst = tst_next

            pool_sb = epsp.tile([B, HID], F32, tag="poolsb")
            nc.vector.tensor_copy(out=pool_sb[:], in_=pool_ps[:])
            nc.sync.dma_start(out=pool_in[:], in_=pool_sb[:])
            nc.gpsimd.collective_compute(
                "AllReduce", mybir.AluOpType.add, replica_groups=groups,
                ins=[pool_in[:]], outs=[pool_out[:]])
            pooled = epsp.tile([B, HID], F32, tag="pooled")
            nc.sync.dma_start(out=pooled[:], in_=pool_out[:])
            pm = epsp.tile([B, HID], F32, tag="pm")
            nc.vector.tensor_tensor(
                out=pm[:], in0=pooled[:],
                in1=invcnt_sb[:].to_broadcast([B, HID]),
                op=mybir.AluOpType.mult)
            ppT = psA.tile([HID, B], F32, tag="psA")
            nc.tensor.matmul(ppT[:], pm[:], identB_sb[:], start=True, stop=True)
            pT = epsp.tile([HID, B], F32, tag="pT")
            nc.vector.tensor_copy(out=pT[:], in_=ppT[:])
            z1p = psB.tile([HID // 2, B], F32, tag="psB")
            nc.tensor.matmul(z1p[:], mlpw_sb[:, :HID // 2], pT[:], start=True, stop=True)
            z1 = epsp.tile([HID // 2, B], F32, tag="z1")
            nc.scalar.activation(
                out=z1[:], in_=z1p[:],
                func=mybir.ActivationFunctionType.Relu, bias=bm_sb[:, 0:1])
            z2p = psB.tile([1, B], F32, tag="psB")
            nc.tensor.matmul(
                z2p[:], mlpw_sb[:HID // 2, HID // 2:HID // 2 + 1], z1[:],
                start=True, stop=True)
            zf = epsp.tile([1, B], F32, tag="zf")
            nc.vector.tensor_tensor(
                out=zf[:], in0=z2p[:],
                in1=bm_sb[0:1, 1:2].to_broadcast([1, B]),
                op=mybir.AluOpType.add)
            nc.sync.dma_start(out=z[:], in_=zf[:])

    nc.finalize()
    return nc


_CACHE = {}


def kernel(x, edge_index, batch, W0, b0, W1, b1, W2, b2, Wm1, bm1, Wm2, bm2,
           trace=False):
    x = np.asarray(x, np.float32)
    dis, meta, idx_stream, dl_stream, dlB, perm = _preprocess(
        np.asarray(edge_index), np.asarray(batch))
    S = meta["S"]
    Ctot = S // 128

    gid = np.asarray(batch, np.int64)
    cnts = np.bincount(gid, minlength=B).astype(np.float32)
    packed = perm

    xp = np.zeros((NCORES, IN_C, NLOC), np.float16)
    xp[packed // NLOC, :, packed % NLOC] = x.astype(np.float16)
    disp = np.zeros((NCORES, NLOC), np.float32)
    disp[packed // NLOC, packed % NLOC] = dis
    seg = np.zeros((NCORES, NLOC, B), np.float16)
    seg[packed // NLOC, packed % NLOC, gid] = 1.0

    iota = np.tile(np.arange(256, dtype=np.float16)[None, :], (128, 1))
    in_maps = []
    for k in range(NCORES):
        in_maps.append(dict(
            xT=xp[k],
            disN=disp[k][:, None].copy(),
            disT=disp[k][None, :].astype(np.float16).copy(),
            W0=np.asarray(W0, np.float16),
            W1=np.asarray(W1, np.float16),
            W2=np.asarray(W2, np.float16),
            b0=np.asarray(b0, np.float32)[:, None],
            b1=np.asarray(b1, np.float32)[:, None],
            b2=np.asarray(b2, np.float32)[:, None],
            Wm1=np.asarray(Wm1, np.float32),
            bm1=np.asarray(bm1, np.float32)[:, None],
            Wm2=np.asarray(Wm2, np.float32),
            bm2=np.asarray(bm2, np.float32).reshape(1, 1),
            idx16=np.tile(idx_stream[k].reshape(S // 16, 16).T, (8, 1)).copy(),
            dl16=dl_stream[k].reshape(Ctot, 128).T.copy(),
            dlB16=dlB[k].T.copy(),
            segsel=seg[k],
            invcnt=(1.0 / np.maximum(cnts, 1.0)).astype(np.float32)[:, None],
            iota128=iota,
            ident64=np.eye(HID, dtype=np.float16),
            ident128=np.eye(128, dtype=np.float16),
            identB=np.eye(B, dtype=np.float32),
        ))

    ckey = (meta["CU"], meta["jobs"].tobytes(), meta["bfirst"].tobytes(),
            meta["done_step"].tobytes(), meta["bnd_ord"].tobytes())
    if ckey not in _CACHE:
        _CACHE[ckey] = _build_nc(meta)
    nc = _CACHE[ckey]

    res = run_bass_kernel_spmd(nc, in_maps, list(range(NCORES)), trace=trace)
    out = res.results[0]["z"].reshape(B, 1).astype(np.float32)
    if trace:
        return out, res
    return out


# revision 46
# speedup vs baseline: 1.0443x; 1.0443x over previous
"""FootballGCN (3x GCNConv + mean-pool + MLP) on 8 TRN2 NeuronCores.

Self-contained: takes full inputs, shards internally, runs a Bass/Tile SPMD
kernel via run_bass_kernel_spmd, returns the full (B, 1) output.

Strategy (dst-sharded message passing, feature-major on-chip layout):
  - nodes packed into 8 cores x 12544 local slots (12500 real + pad)
  - per layer: table t = dis * (h @ W) built per 128-node block via one
    matmul (lhsT = feature-major h block -> node-major psum, no transpose),
    quantized f16, duplicated to 256B rows, AllGathered to every core's HBM
  - edge pass: (g, blk)-major slot stream with core-invariant per-(g,blk)
    slot counts; chunks of 128 slots cut independently of block boundaries;
    per (group, step) one dma_gather of up to CH chunks, with desc-gen
    parallelized across the 4 SWDGE queues (queue_num=g selects the Q7 DSP
    pair in ucode, so 4 gathers' descriptor generation runs concurrently)
  - selection matrices on DVE via is_equal(dl, iota) where
    dl = dcol + 128*(blk != bfirst(chunk)); PE matmul (lhsT=msg, rhs=Sel)
    accumulates agg[64, 128] per block in PSUM; self-loops are folded in as
    one transpose-matmul of the local t block (lhsT=tst, rhs=I128)
  - epilogue relu(dis*agg + b) on DVE+ACT
  - pooling via segsel matmuls, AllReduce, tiny MLP, output z
"""
import numpy as np

import concourse.bass as bass
import concourse.mybir as mybir
import concourse.tile as tile
from concourse import bacc as bacc_mod
from concourse.bass_utils import run_bass_kernel_spmd

F16 = mybir.dt.float16
F32 = mybir.dt.float32
F8 = mybir.dt.float8e4
I16 = mybir.dt.int16

# ---- problem dims (hardcoded per spec) ----
N = 100000
E = 3200000
B = 128
IN_C, HID = 128, 64
NCORES = 8
NREAL = 12500
NLOC = 12544                 # 98 * 128
NBLK = NLOC // 128           # 98
NTOT = NCORES * NLOC         # 100352
NGRP = 4
GRP = NTOT // NGRP           # 25088
NHALF = NLOC // 2            # 6272 rows = blocks 0-48
CH = 17                      # chunks per gather


def _preprocess(edge_index, batch):
    src_g = np.asarray(edge_index[0], np.int64)
    dst_g = np.asarray(edge_index[1], np.int64)
    loops = np.arange(N, dtype=np.int64)
    dst_all = np.concatenate([dst_g, loops])
    deg = np.bincount(dst_all, minlength=N).astype(np.float64)
    dis = (1.0 / np.sqrt(np.maximum(deg, 1.0))).astype(np.float32)

    # snake-balanced node -> packed-slot assignment: deal nodes (sorted by
    # in-degree desc) across all NCORES*NBLK blocks so per-(core,g,blk)
    # edge counts equalize -> less cross-core max padding.
    nblk_all = NCORES * NBLK
    order = np.argsort(-deg, kind="stable")
    pos = np.arange(N)
    cyc, r = pos // nblk_all, pos % nblk_all
    blk_of = np.where(cyc % 2 == 0, r, nblk_all - 1 - r)
    rank_of = cyc
    perm = np.empty(N, dtype=np.int64)
    perm[order] = (blk_of // NBLK) * NLOC + (blk_of % NBLK) * 128 + rank_of
    assert rank_of.max() < 128

    # gather groups are (src table-half, src core-quad): the table is
    # AllGathered in two halves (tfA = local rows [0, NHALF), tfB = rest),
    # so a group must be one contiguous 25088-row window of tfA or tfB.
    # Group membership is invariant under the within-pair LPT below.
    def grp_lidx(p):
        c, loc = p // NLOC, p % NLOC
        g = (loc >= NHALF) * 2 + (c >= 4)
        return g, (c % 4) * NHALF + (loc % NHALF)

    # LPT within each (core-pair, block-col): rebalance nodes between the
    # two cores to equalize per-(g, blk) in-edge counts across cores.
    grp_node0 = grp_lidx(perm)[0]
    vg = np.zeros((N, NGRP), np.int32)
    np.add.at(vg, (dst_g, grp_node0[src_g]), 1)
    core_of = perm // NLOC
    blk_idx = (perm % NLOC) // 128
    for p in range(4):
        for b in range(NBLK):
            sel = np.flatnonzero(
                ((core_of == 2 * p) | (core_of == 2 * p + 1)) & (blk_idx == b))
            v = vg[sel].astype(np.int64)
            o2 = np.argsort(-v.sum(1), kind="stable")
            sumA = np.zeros(NGRP, np.int64)
            sumB = np.zeros(NGRP, np.int64)
            nA = nB = 0
            sideA = np.zeros(len(sel), bool)
            for i in o2:
                mA = np.maximum(sumA + v[i], sumB).max()
                mB = np.maximum(sumA, sumB + v[i]).max()
                if (mA <= mB and nA < 128) or nB >= 128:
                    sideA[i] = True
                    sumA += v[i]
                    nA += 1
                else:
                    sumB += v[i]
                    nB += 1
            ranksA = np.flatnonzero(sideA)
            ranksB = np.flatnonzero(~sideA)
            newpos = np.empty(len(sel), np.int64)
            newpos[ranksA] = 2 * p * NLOC + b * 128 + np.arange(len(ranksA))
            newpos[ranksB] = ((2 * p + 1) * NLOC + b * 128
                              + np.arange(len(ranksB)))
            perm[sel] = newpos

    # self-loops are NOT in the stream (folded into the per-block
    # transpose-matmul of the local t block instead)
    src_p = perm[src_g]
    dst_p = perm[dst_g]

    core = dst_p // NLOC
    blk = (dst_p % NLOC) // 128
    dcol = dst_p % 128
    grp, lidx = grp_lidx(src_p)

    key = (core * NGRP + grp) * NBLK + blk
    cnt = np.bincount(key, minlength=NCORES * NGRP * NBLK).reshape(
        NCORES, NGRP, NBLK)
    m = cnt.max(axis=0)                      # (NGRP, NBLK) core-invariant
    CU = int(np.ceil(m.sum(axis=1).max() / 128))
    SGRP = CU * 128
    S = NGRP * SGRP

    off = np.zeros((NGRP, NBLK), np.int64)
    off[:, 1:] = np.cumsum(m[:, :-1], axis=1)

    # first/last block of each (g, chunk)
    seg_end = np.cumsum(m, axis=1)           # (NGRP, NBLK) slot end per block
    bfirst = np.zeros((NGRP, CU), np.int64)
    blast = np.zeros((NGRP, CU), np.int64)
    for g in range(NGRP):
        bfirst[g] = np.searchsorted(seg_end[g], np.arange(CU) * 128,
                                    side="right")
        bfirst[g] = np.minimum(bfirst[g], NBLK - 1)
        blast[g] = np.searchsorted(seg_end[g], np.arange(1, CU + 1) * 128 - 1,
                                   side="right")
        blast[g] = np.minimum(blast[g], NBLK - 1)

    # boundary chunks (span a block boundary) need a second sel built from
    # dlB = dl - 128; bnd_ord[g, ch] = ordinal into the packed dlB table
    bnd_ord = np.full((NGRP, CU), -1, np.int64)
    nbnd = 0
    for g in range(NGRP):
        for ch in range(CU):
            if bfirst[g, ch] != blast[g, ch]:
                bnd_ord[g, ch] = nbnd
                nbnd += 1

    # slot assignment: sort edges by (core, g, blk, lidx); lidx order gives
    # ascending HBM addresses within each run for better row locality
    o = np.lexsort((lidx, blk, grp, core))
    core_s, grp_s, blk_s, dcol_s, lidx_s = (
        core[o], grp[o], blk[o], dcol[o], lidx[o])
    keyo = (core_s * NGRP + grp_s) * NBLK + blk_s
    is_start = np.ones(len(keyo), bool)
    is_start[1:] = keyo[1:] != keyo[:-1]
    run_start = np.flatnonzero(is_start)
    run_id = np.cumsum(is_start) - 1
    run_pos = np.arange(len(keyo)) - run_start[run_id]
    slot = grp_s * SGRP + off[grp_s, blk_s] + run_pos

    idx_stream = np.zeros((NCORES, S), np.int16)
    dl_stream = np.full((NCORES, S), -1.0, np.float16)
    idx_stream[core_s, slot] = lidx_s.astype(np.int16)
    chunk_of = (slot % SGRP) // 128
    second = (blk_s != bfirst[grp_s, chunk_of]).astype(np.int64)
    dl_stream[core_s, slot] = (dcol_s + 128 * second).astype(np.float16)

    # per-block job chunk ranges (group-local chunk indices)
    jobs = np.zeros((NBLK, NGRP, 2), np.int64)
    for b in range(NBLK):
        for g in range(NGRP):
            s0, s1 = off[g, b], off[g, b] + m[g, b]
            jobs[b, g, 0] = s0 // 128
            jobs[b, g, 1] = (s1 + 127) // 128

    NSTEP = (CU + CH - 1) // CH
    done_step = np.zeros(NBLK, np.int64)
    for b in range(NBLK):
        endc = jobs[b, :, 1].max()
        done_step[b] = (int(endc) + CH - 1) // CH - 1 if endc > 0 else 0

    # host-packed dl tables: dl per chunk, and dl-128 for boundary chunks
    dl_cols = dl_stream.reshape(NCORES, S // 128, 128)  # (cores, Ctot, 128)
    dlB = np.zeros((NCORES, max(nbnd, 1), 128), np.float16)
    for g in range(NGRP):
        for ch in range(CU):
            j = bnd_ord[g, ch]
            if j >= 0:
                dlB[:, j, :] = (
                    dl_cols[:, g * CU + ch, :].astype(np.float32) - 128.0
                ).astype(np.float16)

    meta = dict(CU=CU, S=S, jobs=jobs, bfirst=bfirst, NSTEP=NSTEP,
                done_step=done_step, bnd_ord=bnd_ord, nbnd=max(nbnd, 1))
    return dis, meta, idx_stream, dl_stream, dlB, perm


def _build_nc(meta):
    CU, S = meta["CU"], meta["S"]
    jobs, bfirst = meta["jobs"], meta["bfirst"]
    NSTEP, done_step = meta["NSTEP"], meta["done_step"]
    bnd_ord, NBND = meta["bnd_ord"], meta["nbnd"]
    SGRP = CU * 128
    Ctot = S // 128

    # per (g, step): [first boundary ordinal, count] in the step's chunk range
    bnd_rng = np.zeros((NGRP, NSTEP, 2), np.int64)
    for g in range(NGRP):
        for st in range(NSTEP):
            ords = [bnd_ord[g, ch]
                    for ch in range(st * CH, min((st + 1) * CH, CU))
                    if bnd_ord[g, ch] >= 0]
            if ords:
                bnd_rng[g, st] = (ords[0], len(ords))

    # 4 SWDGE queues: dma_gather desc-gen runs on the Q7 DSP pair selected
    # by queue_num (ucode: cpu_id/2 == queue_num), so the 4 per-step group
    # gathers generate descriptors concurrently on disjoint DSP pairs.
    nc = bacc_mod.Bacc(num_swdge_queues=4)

    xT = nc.declare_dram_parameter("xT", [IN_C, NLOC], F16, isOutput=False)
    disN = nc.declare_dram_parameter("disN", [NLOC, 1], F32, isOutput=False)
    disT = nc.declare_dram_parameter("disT", [1, NLOC], F16, isOutput=False)
    W0 = nc.declare_dram_parameter("W0", [IN_C, HID], F16, isOutput=False)
    W1 = nc.declare_dram_parameter("W1", [HID, HID], F16, isOutput=False)
    W2 = nc.declare_dram_parameter("W2", [HID, HID], F16, isOutput=False)
    b0 = nc.declare_dram_parameter("b0", [HID, 1], F32, isOutput=False)
    b1 = nc.declare_dram_parameter("b1", [HID, 1], F32, isOutput=False)
    b2 = nc.declare_dram_parameter("b2", [HID, 1], F32, isOutput=False)
    Wm1 = nc.declare_dram_parameter("Wm1", [HID, HID // 2], F32, isOutput=False)
    bm1 = nc.declare_dram_parameter("bm1", [HID // 2, 1], F32, isOutput=False)
    Wm2 = nc.declare_dram_parameter("Wm2", [HID // 2, 1], F32, isOutput=False)
    bm2 = nc.declare_dram_parameter("bm2", [1, 1], F32, isOutput=False)
    idx16 = nc.declare_dram_parameter("idx16", [128, S // 16], I16, isOutput=False)
    dl16 = nc.declare_dram_parameter("dl16", [128, Ctot], F16, isOutput=False)
    dlB16 = nc.declare_dram_parameter("dlB16", [128, NBND], F16, isOutput=False)
    segsel = nc.declare_dram_parameter("segsel", [NLOC, B], F16, isOutput=False)
    invcnt = nc.declare_dram_parameter("invcnt", [B, 1], F32, isOutput=False)
    iota128 = nc.declare_dram_parameter("iota128", [128, 256], F16, isOutput=False)
    ident64 = nc.declare_dram_parameter("ident64", [HID, HID], F16, isOutput=False)
    ident128 = nc.declare_dram_parameter("ident128", [128, 128], F16, isOutput=False)
    identB = nc.declare_dram_parameter("identB", [B, B], F32, isOutput=False)
    z = nc.declare_dram_parameter("z", [1, B], F32, isOutput=True)

    # half-table staging + AllGather outputs, double-buffered by layer parity
    t_lA = [nc.dram_tensor(f"t_lA{p}", [NHALF, 128], F16) for p in range(2)]
    t_lB = [nc.dram_tensor(f"t_lB{p}", [NHALF, 128], F16) for p in range(2)]
    tfA = [nc.dram_tensor(f"tfA{p}", [NCORES * NHALF, 128], F16,
                          addr_space="Shared") for p in range(2)]
    tfB = [nc.dram_tensor(f"tfB{p}", [NCORES * NHALF, 128], F16,
                          addr_space="Shared") for p in range(2)]
    pool_in = nc.dram_tensor("pool_in", [B, HID], F32)
    pool_out = nc.dram_tensor("pool_out", [B, HID], F32, addr_space="Shared")

    groups = [list(range(NCORES))]

    with tile.TileContext(nc) as tc:
        with (
            tc.tile_pool(name="const", bufs=1) as constp,
            tc.tile_pool(name="hT", bufs=1) as hTp,
            tc.tile_pool(name="tstage", bufs=1) as tstp,
            tc.tile_pool(name="xblk", bufs=3) as xblkp,
            tc.tile_pool(name="idx", bufs=2) as idxp,
            tc.tile_pool(name="msg", bufs=3) as msgp,
            tc.tile_pool(name="sel", bufs=3) as selp,
            tc.tile_pool(name="eps", bufs=3) as epsp,
            tc.tile_pool(name="psA", bufs=2, space="PSUM") as psA,
            tc.tile_pool(name="psB", bufs=2, space="PSUM") as psB,
            tc.tile_pool(name="psPool", bufs=1, space="PSUM") as psPoolp,
        ):
            disN_sb = constp.tile([128, NBLK], F32)
            nc.sync.dma_start(
                out=disN_sb[:], in_=disN.rearrange("(b p) o -> p (b o)", p=128))
            disTfull = constp.tile([HID, NLOC], F16)
            nc.sync.dma_start(
                out=disTfull[:].unsqueeze(1),
                in_=disT[:, :].partition_broadcast(HID))
            W0_sb = constp.tile([IN_C, HID], F16)
            nc.sync.dma_start(out=W0_sb[:], in_=W0[:])
            xfull = constp.tile([IN_C, NLOC], F16)
            nc.sync.dma_start(out=xfull[:], in_=xT[:])
            W1_sb = constp.tile([HID, HID], F16)
            nc.sync.dma_start(out=W1_sb[:], in_=W1[:])
            W2_sb = constp.tile([HID, HID], F16)
            nc.sync.dma_start(out=W2_sb[:], in_=W2[:])
            bias_sb = constp.tile([HID, 3], F32)
            for i, bb in enumerate([b0, b1, b2]):
                nc.sync.dma_start(out=bias_sb[:, i:i + 1], in_=bb[:])
            iota_sb = constp.tile([128, 256], F16)
            nc.sync.dma_start(out=iota_sb[:], in_=iota128[:])
            dl_sb = constp.tile([128, Ctot], F16)
            nc.sync.dma_start(out=dl_sb[:], in_=dl16[:])
            dlB_sb = constp.tile([128, NBND], F16)
            nc.sync.dma_start(out=dlB_sb[:], in_=dlB16[:])
            ident64_sb = constp.tile([HID, HID], F16)
            nc.sync.dma_start(out=ident64_sb[:], in_=ident64[:])
            ident128_sb = constp.tile([128, 128], F16)
            nc.sync.dma_start(out=ident128_sb[:], in_=ident128[:])
            identB_sb = constp.tile([B, B], F32)
            nc.sync.dma_start(out=identB_sb[:], in_=identB[:])
            invcnt_sb = constp.tile([B, 1], F32)
            nc.sync.dma_start(out=invcnt_sb[:], in_=invcnt[:])
            mlpw_sb = constp.tile([HID, HID // 2 + 1], F32)
            nc.sync.dma_start(out=mlpw_sb[:, :HID // 2], in_=Wm1[:])
            nc.sync.dma_start(out=mlpw_sb[:HID // 2, HID // 2:], in_=Wm2[:])
            bm_sb = constp.tile([HID // 2, 2], F32)
            nc.sync.dma_start(out=bm_sb[:, 0:1], in_=bm1[:])
            nc.sync.dma_start(out=bm_sb[0:1, 1:2], in_=bm2[:])

            hT = hTp.tile([HID, NLOC], F16, tag="hT")

            def stage_ag(par, half, tsrc):
                # stage one table half (duplicated to 256B rows) + AllGather
                t_l = (t_lA if half == 0 else t_lB)[par]
                tf = (tfA if half == 0 else tfB)[par]
                tdst = t_l.rearrange("(b p) e -> p b e", p=128)
                b0 = half * (NBLK // 2)
                nc.sync.dma_start(
                    out=tdst[:, :, 0:HID], in_=tsrc[:, b0:b0 + NBLK // 2, :])
                nc.sync.dma_start(
                    out=tdst[:, :, HID:128], in_=tsrc[:, b0:b0 + NBLK // 2, :])
                nc.gpsimd.collective_compute(
                    "AllGather", mybir.AluOpType.bypass, replica_groups=groups,
                    ins=[t_l[:]], outs=[tf[:]])

            # layer-0 t-table upfront from x; first-half AllGather fires at
            # the midpoint so it overlaps the rest of the build
            tst = tstp.tile([128, NBLK, HID], F16, tag="tstage")
            for b in range(NBLK):
                pt = psA.tile([128, HID], F32, tag="psA")
                nc.tensor.matmul(
                    pt[:], xfull[:, b * 128:(b + 1) * 128], W0_sb[:],
                    start=True, stop=True)
                nc.vector.tensor_tensor(
                    out=tst[:, b, :], in0=pt[:],
                    in1=disN_sb[:, b:b + 1].to_broadcast([128, HID]),
                    op=mybir.AluOpType.mult)
                if b == NBLK // 2 - 1:
                    stage_ag(0, 0, tst)
            stage_ag(0, 1, tst)

            for l in range(3):
                par = l % 2
                tst_next = None
                if l < 2:
                    tst_next = tstp.tile([128, NBLK, HID], F16, tag="tstage")
                else:
                    pool_ps = psPoolp.tile([B, HID], F32)
                    segd = segsel.rearrange("(b p) g -> p b g", p=128)
                Wn_sb = W1_sb if l == 0 else W2_sb

                mts = [[None] * NSTEP for _ in range(NGRP)]
                selAs = [[None] * NSTEP for _ in range(NGRP)]
                selBs = [[None] * NSTEP for _ in range(NGRP)]
                nb = 0
                # issue order: tfA-group gathers of the first PRE steps go
                # first — tfA was AllGathered mid-previous-layer, so these
                # run while this layer's AG2 (tfB) is still completing.
                # PRE=3 needs sel/msg pools at bufs=3 so the three steps of
                # tiles issued ahead of their readers stay within rotation.
                PRE = 3
                plan = ([(s, g) for s in range(min(PRE, NSTEP))
                         for g in range(2)]
                        + [(s, g) for s in range(min(PRE, NSTEP))
                           for g in range(2, NGRP)]
                        + [(s, g) for s in range(PRE, NSTEP)
                           for g in range(NGRP)])
                issued = [0] * NSTEP
                scomp = -1
                for (step, g) in plan:
                    c0 = step * CH
                    c1 = min((step + 1) * CH, CU)
                    nch = c1 - c0
                    sl0 = g * SGRP + c0 * 128
                    nidx = nch * 128
                    idxb = idxp.tile([128, nidx // 16], I16, tag=f"idx{g}")
                    nc.sync.dma_start(
                        out=idxb[:],
                        in_=idx16[:, sl0 // 16:(sl0 + nidx) // 16])
                    mt = msgp.tile([128, nch, 128], F16, tag=f"msg{g}")
                    tf_g = (tfA if g < 2 else tfB)[par]
                    goff = (g % 2) * GRP
                    nc.gpsimd.dma_gather(
                        out_ap=mt[:],
                        in_ap=tf_g[goff:goff + GRP, :],
                        idxs_ap=idxb[:],
                        num_idxs=nidx, num_idxs_reg=nidx, elem_size=128,
                        single_packet=False, queue_num=g)
                    mts[g][step] = mt
                    selA = selp.tile([128, nch, 128], F8, tag=f"selA{g}")
                    nc.vector.tensor_tensor(
                        out=selA[:],
                        in0=dl_sb[:, g * CU + c0:g * CU + c1]
                        .unsqueeze(2).to_broadcast([128, nch, 128]),
                        in1=iota_sb[:, 0:128].unsqueeze(1)
                        .to_broadcast([128, nch, 128]),
                        op=mybir.AluOpType.is_equal)
                    selAs[g][step] = selA
                    o0, nbd = int(bnd_rng[g, step, 0]), int(bnd_rng[g, step, 1])
                    if nbd > 0:
                        selB = selp.tile([128, nbd, 128], F8, tag=f"selB{g}")
                        nc.vector.tensor_tensor(
                            out=selB[:],
                            in0=dlB_sb[:, o0:o0 + nbd]
                            .unsqueeze(2).to_broadcast([128, nbd, 128]),
                            in1=iota_sb[:, 0:128].unsqueeze(1)
                            .to_broadcast([128, nbd, 128]),
                            op=mybir.AluOpType.is_equal)
                        selBs[g][step] = (selB, o0)

                    issued[step] += 1
                    while (scomp + 1 < NSTEP
                           and issued[scomp + 1] == NGRP):
                        scomp += 1
                    while nb < NBLK and done_step[nb] <= scomp:
                        b = nb
                        nb += 1
                        agg = psB.tile([HID, 128], F32, tag="psB")
                        first = True
                        for g in range(NGRP):
                            jc0, jc1 = int(jobs[b, g, 0]), int(jobs[b, g, 1])
                            for ch in range(jc0, jc1):
                                st, col = ch // CH, ch % CH
                                if bfirst[g, ch] == b:
                                    stile = selAs[g][st][:, col, :]
                                else:
                                    sB, o0 = selBs[g][st]
                                    stile = sB[:, int(bnd_ord[g, ch]) - o0, :]
                                nc.tensor.matmul(
                                    agg[:], mts[g][st][:, col, 0:HID],
                                    stile,
                                    start=first, stop=False)
                                first = False
                        # self-loop: agg[f, n] += t_blk[n, f] via transpose
                        nc.tensor.matmul(
                            agg[:], tst[:, b, :], ident128_sb[:],
                            start=first, stop=True)
                        ep = epsp.tile([HID, 128], F32, tag="eps")
                        nc.vector.tensor_tensor(
                            out=ep[:], in0=agg[:],
                            in1=disTfull[:, b * 128:(b + 1) * 128],
                            op=mybir.AluOpType.mult)
                        nc.scalar.activation(
                            out=hT[:, b * 128:(b + 1) * 128], in_=ep[:],
                            func=mybir.ActivationFunctionType.Relu,
                            bias=bias_sb[:, l:l + 1])
                        if l < 2:
                            # next layer's t for this block, overlapped here
                            pt = psA.tile([128, HID], F32, tag="psA")
                            nc.tensor.matmul(
                                pt[:], hT[:, b * 128:(b + 1) * 128], Wn_sb[:],
                                start=True, stop=True)
                            nc.vector.tensor_tensor(
                                out=tst_next[:, b, :], in0=pt[:],
                                in1=disN_sb[:, b:b + 1].to_broadcast([128, HID]),
                                op=mybir.AluOpType.mult)
                            if b == NBLK // 2 - 1:
                                # first-half AllGather for layer l+1 fires
                                # mid-layer and hides under the edge pass
                                stage_ag(1 - par, 0, tst_next)
                        else:
                            # pooling contribution, overlapped here
                            h3t = psA.tile([128, HID], F32, tag="psA")
                            nc.tensor.matmul(
                                h3t[:], hT[:, b * 128:(b + 1) * 128],
                                ident64_sb[:], start=True, stop=True)
                            h3s = epsp.tile([128, HID], F16, tag="h3s")
                            nc.vector.tensor_copy(out=h3s[:], in_=h3t[:])
                            segb = xblkp.tile([128, B], F16, tag="segb")
                            nc.sync.dma_start(out=segb[:], in_=segd[:, b, :])
                            nc.tensor.matmul(
                                pool_ps[:], segb[:], h3s[:],
                                start=(b == 0), stop=(b == NBLK - 1))
                assert nb == NBLK
                if l < 2:
                    stage_ag(1 - par, 1, tst_next)
                    tst = tst_next

            pool_sb = epsp.tile([B, HID], F32, tag="poolsb")
            nc.vector.tensor_copy(out=pool_sb[:], in_=pool_ps[:])
            nc.sync.dma_start(out=pool_in[:], in_=pool_sb[:])
            nc.gpsimd.collective_compute(
                "AllReduce", mybir.AluOpType.add, replica_groups=groups,
                ins=[pool_in[:]], outs=[pool_out[:]])
            pooled = epsp.tile([B, HID], F32, tag="pooled")
            nc.sync.dma_start(out=pooled[:], in_=pool_out[:])
            pm = epsp.tile([B, HID], F32, tag="pm")
            nc.vector.tensor_tensor(
                out=pm[:], in0=pooled[:],
                in1=invcnt_sb[:].to_broadcast([B, HID]),
                op=mybir.AluOpType.mult)
            ppT = psA.tile([HID, B], F32, tag="psA")
            nc.tensor.matmul(ppT[:], pm[:], identB_sb[:], start=True, stop=True)
            pT = epsp.tile([HID, B], F32, tag="pT")
            nc.vector.tensor_copy(out=pT[:], in_=ppT[:])
            z1p = psB.tile([HID // 2, B], F32, tag="psB")
            nc.tensor.matmul(z1p[:], mlpw_sb[:, :HID // 2], pT[:], start=True, stop=True)
            z1 = epsp.tile([HID // 2, B], F32, tag="z1")
            nc.scalar.activation(
                out=z1[:], in_=z1p[:],
                func=mybir.ActivationFunctionType.Relu, bias=bm_sb[:, 0:1])
            z2p = psB.tile([1, B], F32, tag="psB")
            nc.tensor.matmul(
                z2p[:], mlpw_sb[:HID // 2, HID // 2:HID // 2 + 1], z1[:],
                start=True, stop=True)
            zf = epsp.tile([1, B], F32, tag="zf")
            nc.vector.tensor_tensor(
                out=zf[:], in0=z2p[:],
                in1=bm_sb[0:1, 1:2].to_broadcast([1, B]),
                op=mybir.AluOpType.add)
            nc.sync.dma_start(out=z[:], in_=zf[:])

    nc.finalize()
    return nc


_CACHE = {}


def kernel(x, edge_index, batch, W0, b0, W1, b1, W2, b2, Wm1, bm1, Wm2, bm2,
           trace=False):
    x = np.asarray(x, np.float32)
    dis, meta, idx_stream, dl_stream, dlB, perm = _preprocess(
        np.asarray(edge_index), np.asarray(batch))
    S = meta["S"]
    Ctot = S // 128

    gid = np.asarray(batch, np.int64)
    cnts = np.bincount(gid, minlength=B).astype(np.float32)
    packed = perm

    xp = np.zeros((NCORES, IN_C, NLOC), np.float16)
    xp[packed // NLOC, :, packed % NLOC] = x.astype(np.float16)
    disp = np.zeros((NCORES, NLOC), np.float32)
    disp[packed // NLOC, packed % NLOC] = dis
    seg = np.zeros((NCORES, NLOC, B), np.float16)
    seg[packed // NLOC, packed % NLOC, gid] = 1.0

    iota = np.tile(np.arange(256, dtype=np.float16)[None, :], (128, 1))
    in_maps = []
    for k in range(NCORES):
        in_maps.append(dict(
            xT=xp[k],
            disN=disp[k][:, None].copy(),
            disT=disp[k][None, :].astype(np.float16).copy(),
            W0=np.asarray(W0, np.float16),
            W1=np.asarray(W1, np.float16),
            W2=np.asarray(W2, np.float16),
            b0=np.asarray(b0, np.float32)[:, None],
            b1=np.asarray(b1, np.float32)[:, None],
            b2=np.asarray(b2, np.float32)[:, None],
            Wm1=np.asarray(Wm1, np.float32),
            bm1=np.asarray(bm1, np.float32)[:, None],
            Wm2=np.asarray(Wm2, np.float32),
            bm2=np.asarray(bm2, np.float32).reshape(1, 1),
            idx16=np.tile(idx_stream[k].reshape(S // 16, 16).T, (8, 1)).copy(),
            dl16=dl_stream[k].reshape(Ctot, 128).T.copy(),
            dlB16=dlB[k].T.copy(),
            segsel=seg[k],
            invcnt=(1.0 / np.maximum(cnts, 1.0)).astype(np.float32)[:, None],
            iota128=iota,
            ident64=np.eye(HID, dtype=np.float16),
            ident128=np.eye(128, dtype=np.float16),
            identB=np.eye(B, dtype=np.float32),
        ))

    ckey = (meta["CU"], meta["jobs"].tobytes(), meta["bfirst"].tobytes(),
            meta["done_step"].tobytes(), meta["bnd_ord"].tobytes())
    if ckey not in _CACHE:
        _CACHE[ckey] = _build_nc(meta)
    nc = _CACHE[ckey]

    res = run_bass_kernel_spmd(nc, in_maps, list(range(NCORES)), trace=trace)
    out = res.results[0]["z"].reshape(B, 1).astype(np.float32)
    if trace:
        return out, res
    return out
